# revision 10
# baseline (speedup 1.0000x reference)
"""KAN B-spline activation kernel for Trainium2 (8 NeuronCores, data-parallel on batch).

Math (validated vs reference to ~1e-7 rel):
  grid is uniform: g[t] = -1 + (t-3)*h, h = 0.125, t = 0..22; u = (x - g[0])/h = 8x + 11.
  For x in [0,1) only knot-window t in [8,18] has nonzero cubic bases.
  Let A[k] = x - g[8+k], k = 0..15 (k=15 unused pad).
  B1[m]  = Relu(1 - |A[m+1]|/h)                     (hat; == Cox-de Boor level 1), m=0..12
  B2d[m] = A[m]*B1[m]   - A[m+3]*B1[m+1]           (= 2h * B2), m=0..11
  B3d[m] = A[m]*B2d[m]  - A[m+4]*B2d[m+1]          (= 6h^2 * B3), m=0..10
  out[b,o,i] = sum_m B3d[b,i,m] * coef[o,i,8+m] / (6 h^2)

Device layout (per core, 128 batch rows in partitions):
  A/B* tiles: (128, 64 inputs x 16-knot-window blocks) in the free dim.
  B3 (128, 64*16) -> 8 PE transposes of 128-col groups (8 inputs each) ->
  basesT (K=(input,knot) partitions, batch free). Matmul per (group g, subgroup s):
  K=32 (2 inputs x 16 knots), N=128 (2 inputs x 64 outs), block-diagonal rhs built
  on host with the zeros/padding baked in. PSUM bank per group (128,512) is copied
  verbatim to SBUF and DMA'd out; host un-permutes (b, g, s, p, o) -> (b, o, i).
"""

import numpy as np
from contextlib import ExitStack

import concourse.bass as bass
import concourse.tile as tile
from concourse import bacc, mybir
from concourse.bass_utils import run_bass_kernel_spmd
from concourse.masks import make_identity

N_CORES = 8
B_TOT, IN_DIM, OUT_DIM = 1024, 64, 64
BPC = B_TOT // N_CORES          # 128 batch rows per core
K16 = 16                        # padded knot window per input
NG = 8                          # groups of 8 inputs
F32 = mybir.dt.float32

# If the stride-0 broadcast read on DVE fails, flip to False (log-doubling copies).
# HW faulted with stride-0 input APs on DVE (sim accepts them); use doubling.
USE_STRIDE0 = False

_CACHE = {}


def _build_nc():
    nc = bacc.Bacc("TRN2", target_bir_lowering=False, debug=False,
                   num_devices=N_CORES)
    x_d = nc.dram_tensor("x_in", [BPC, IN_DIM], F32, kind="ExternalInput").ap()
    rhs_d = nc.dram_tensor("rhs_in", [128, NG * 512], F32, kind="ExternalInput").ap()
    g3_d = nc.dram_tensor("g3_in", [1, IN_DIM * K16], F32, kind="ExternalInput").ap()
    out_d = nc.dram_tensor("out", [BPC, NG, 512], F32, kind="ExternalOutput").ap()

    with tile.TileContext(nc) as tc, ExitStack() as ctx:
        pool = ctx.enter_context(tc.tile_pool(name="main", bufs=1))
        psT = ctx.enter_context(tc.tile_pool(name="psT", bufs=2, space="PSUM"))
        psO = ctx.enter_context(tc.tile_pool(name="psO", bufs=4, space="PSUM"))
        og_pool = ctx.enter_context(tc.tile_pool(name="og", bufs=4))

        ident = pool.tile([128, 128], F32)
        make_identity(nc, ident)

        x_sb = pool.tile([BPC, IN_DIM], F32)
        nc.sync.dma_start(out=x_sb[:], in_=x_d)
        rhs_sb = pool.tile([128, NG * 512], F32)
        nc.sync.dma_start(out=rhs_sb[:], in_=rhs_d)
        # broadcast the (1, 1024) knot row across 128 partitions during DMA
        g3_sb = pool.tile([128, IN_DIM * K16], F32)
        g3_bcast = bass.AP(tensor=g3_d.tensor, offset=g3_d.offset,
                           ap=[[0, 128]] + list(g3_d.ap[1:]))
        nc.gpsimd.dma_start(out=g3_sb[:], in_=g3_bcast)
        g3v = g3_sb[:].rearrange("p (i k) -> p i k", k=K16)

        A = pool.tile([BPC, IN_DIM, K16], F32)
        if USE_STRIDE0:
            xs = x_sb[:]
            x_b = bass.AP(tensor=xs.tensor, offset=xs.offset,
                          ap=[list(xs.ap[0]), list(xs.ap[1]), [0, K16]])
            nc.vector.tensor_sub(A[:], x_b, g3v)
        else:
            xt = pool.tile([BPC, IN_DIM, K16], F32)
            nc.vector.tensor_copy(xt[:, :, 0:1], x_sb[:].rearrange("p (i k) -> p i k", k=1))
            w = 1
            while w < K16:
                n = min(w, K16 - w)
                nc.vector.tensor_copy(xt[:, :, w:w + n], xt[:, :, 0:n])
                w += n
            nc.vector.tensor_sub(A[:], xt[:], g3v)

        Babs = pool.tile([BPC, IN_DIM, 13], F32)
        nc.scalar.activation(out=Babs[:], in_=A[:, :, 1:14],
                             func=mybir.ActivationFunctionType.Abs)
        B1 = pool.tile([BPC, IN_DIM, 13], F32)
        # Relu(-8*|A| + 1) == Relu(1 - |A|/h)
        nc.scalar.activation(out=B1[:], in_=Babs[:],
                             func=mybir.ActivationFunctionType.Relu,
                             scale=-8.0, bias=1.0)

        Ml2 = pool.tile([BPC, IN_DIM, 12], F32)
        Mr2 = pool.tile([BPC, IN_DIM, 12], F32)
        B2 = pool.tile([BPC, IN_DIM, 12], F32)
        nc.vector.tensor_mul(Ml2[:], A[:, :, 0:12], B1[:, :, 0:12])
        nc.gpsimd.tensor_mul(Mr2[:], A[:, :, 3:15], B1[:, :, 1:13])
        nc.vector.tensor_sub(B2[:], Ml2[:], Mr2[:])

        Ml3 = pool.tile([BPC, IN_DIM, 11], F32)
        Mr3 = pool.tile([BPC, IN_DIM, 11], F32)
        B3 = pool.tile([BPC, IN_DIM, K16], F32)
        nc.vector.tensor_mul(Ml3[:], A[:, :, 0:11], B2[:, :, 0:11])
        nc.gpsimd.tensor_mul(Mr3[:], A[:, :, 4:15], B2[:, :, 1:12])
        # pad knots 11..15 must be 0: they face garbage-free rhs rows but feed
        # the transpose, whose output multiplies real coef columns.
        nc.gpsimd.memset(B3[:, :, 11:16], 0.0)
        nc.vector.tensor_sub(B3[:, :, 0:11], Ml3[:], Mr3[:])

        B3f = B3[:].rearrange("p i k -> p (i k)")
        basesT = pool.tile([128, NG * 128], F32)
        for G in range(2):
            ps_t = psT.tile([128, 512], F32)
            for q in range(4):
                g = G * 4 + q
                nc.tensor.transpose(out=ps_t[:, q * 128:(q + 1) * 128],
                                    in_=B3f[:, g * 128:(g + 1) * 128],
                                    identity=ident[:])
            dst = basesT[:, G * 512:(G + 1) * 512]
            if G == 0:
                nc.vector.tensor_copy(dst, ps_t[:])
            else:
                nc.scalar.copy(dst, ps_t[:])

        for g in range(NG):
            ps_o = psO.tile([128, 512], F32)
            nc.tensor.matmul(out=ps_o[:],
                             lhsT=basesT[:, g * 128:(g + 1) * 128],
                             rhs=rhs_sb[:, g * 512:(g + 1) * 512],
                             start=True, stop=True)
            og = og_pool.tile([128, 512], F32)
            if g % 2 == 0:
                nc.vector.tensor_copy(og[:], ps_o[:])
            else:
                nc.scalar.copy(og[:], ps_o[:])
            nc.sync.dma_start(out=out_d[:, g, :], in_=og[:])

    nc.compile()
    return nc


def _host_inputs(x, coef, grid):
    x = np.ascontiguousarray(np.asarray(x, dtype=np.float32))
    coef = np.asarray(coef, dtype=np.float32)
    knots = np.asarray(grid, dtype=np.float32)[0, 0, :]          # (23,)
    h = float(knots[1] - knots[0])

    g3 = np.empty(K16, dtype=np.float32)
    g3[:15] = knots[8:23]
    g3[15] = knots[22] + h                                       # unused pad
    g3row = np.tile(g3, IN_DIM)[None, :]                         # (1, 1024)

    scale = 1.0 / (6.0 * h * h)
    cf = coef[:, :, 8:19] * scale                                # (o, i, 11)
    # block-diagonal rhs per group: rows (i_l,j) x cols (i_l', o), K=128, N=512
    rhs = np.zeros((128, NG * 512), dtype=np.float32)
    for i_l in range(8):
        for g in range(NG):
            i = g * 8 + i_l
            rhs[i_l * 16:i_l * 16 + 11,
                g * 512 + i_l * 64:g * 512 + i_l * 64 + 64] = cf[:, i, :].T
    return x, rhs, g3row


def _execute(x, coef, grid, trace=False, **spmd_kwargs):
    xf, rhs, g3row = _host_inputs(x, coef, grid)
    if "nc" not in _CACHE:
        _CACHE["nc"] = _build_nc()
    nc = _CACHE["nc"]
    in_maps = [{"x_in": np.ascontiguousarray(xf[c * BPC:(c + 1) * BPC]),
                "rhs_in": rhs, "g3_in": g3row} for c in range(N_CORES)]
    res = run_bass_kernel_spmd(nc, in_maps, list(range(N_CORES)),
                               trace=trace, **spmd_kwargs)
    full = np.empty((B_TOT, OUT_DIM, IN_DIM), dtype=np.float32)
    for c in range(N_CORES):
        t = res.results[c]["out"].reshape(BPC, NG, 8, 64)        # (b, g, i_l, o)
        full[c * BPC:(c + 1) * BPC] = (
            t.transpose(0, 3, 1, 2).reshape(BPC, OUT_DIM, IN_DIM))
    return full, res


def kernel(x, coef, grid):
    out, _ = _execute(x, coef, grid, trace=False)
    return out


# revision 12
# speedup vs baseline: 1.1271x; 1.1271x over previous
"""KAN B-spline activation kernel for Trainium2 (8 NeuronCores, data-parallel on batch).

Math (validated vs reference to ~1e-7 rel):
  grid is uniform: g[t] = -1 + (t-3)*h, h = 0.125, t = 0..22; u = (x - g[0])/h = 8x + 11.
  For x in [0,1) only knot-window t in [8,18] has nonzero cubic bases.
  Let A[k] = x - g[8+k], k = 0..15 (k=15 unused pad).
  B1[m]  = Relu(1 - |A[m+1]|/h)                     (hat; == Cox-de Boor level 1), m=0..12
  B2d[m] = A[m]*B1[m]   - A[m+3]*B1[m+1]           (= 2h * B2), m=0..11
  B3d[m] = A[m]*B2d[m]  - A[m+4]*B2d[m+1]          (= 6h^2 * B3), m=0..10
  out[b,o,i] = sum_m B3d[b,i,m] * coef[o,i,8+m] / (6 h^2)

Device layout (per core, 128 batch rows in partitions):
  A/B* tiles: (128, 64 inputs x 16-knot-window blocks) in the free dim.
  B3 (128, 64*16) -> 8 PE transposes of 128-col groups (8 inputs each) ->
  basesT (K=(input,knot) partitions, batch free). Matmul per (group g, subgroup s):
  K=32 (2 inputs x 16 knots), N=128 (2 inputs x 64 outs), block-diagonal rhs built
  on host with the zeros/padding baked in. PSUM bank per group (128,512) is copied
  verbatim to SBUF and DMA'd out; host un-permutes (b, g, s, p, o) -> (b, o, i).
"""

import numpy as np
from contextlib import ExitStack

import concourse.bass as bass
import concourse.tile as tile
from concourse import bacc, mybir
from concourse.bass_utils import run_bass_kernel_spmd
from concourse.masks import make_identity

N_CORES = 8
B_TOT, IN_DIM, OUT_DIM = 1024, 64, 64
BPC = B_TOT // N_CORES          # 128 batch rows per core
K16 = 16                        # padded knot window per input
NG = 8                          # groups of 8 inputs
F32 = mybir.dt.float32

# If the stride-0 broadcast read on DVE fails, flip to False (log-doubling copies).
# HW faulted with stride-0 input APs on DVE (sim accepts them); use doubling.
USE_STRIDE0 = False

_CACHE = {}


def _build_nc():
    nc = bacc.Bacc("TRN2", target_bir_lowering=False, debug=False,
                   num_devices=N_CORES)
    x_d = nc.dram_tensor("x_in", [BPC, IN_DIM], F32, kind="ExternalInput").ap()
    rhs_d = nc.dram_tensor("rhs_in", [128, NG * 512], F32, kind="ExternalInput").ap()
    g3_d = nc.dram_tensor("g3_in", [1, IN_DIM * K16], F32, kind="ExternalInput").ap()
    out_d = nc.dram_tensor("out", [BPC, NG, 512], F32, kind="ExternalOutput").ap()

    with tile.TileContext(nc) as tc, ExitStack() as ctx:
        pool = ctx.enter_context(tc.tile_pool(name="main", bufs=1))
        psT = ctx.enter_context(tc.tile_pool(name="psT", bufs=2, space="PSUM"))
        psO = ctx.enter_context(tc.tile_pool(name="psO", bufs=4, space="PSUM"))
        og_pool = ctx.enter_context(tc.tile_pool(name="og", bufs=4))

        ident = pool.tile([128, 128], F32)
        make_identity(nc, ident)

        x_sb = pool.tile([BPC, IN_DIM], F32)
        nc.sync.dma_start(out=x_sb[:], in_=x_d)
        rhs_sb = pool.tile([128, NG * 512], F32)
        nc.sync.dma_start(out=rhs_sb[:], in_=rhs_d)
        # broadcast the (1, 1024) knot row across 128 partitions during DMA
        g3_sb = pool.tile([128, IN_DIM * K16], F32)
        g3_bcast = bass.AP(tensor=g3_d.tensor, offset=g3_d.offset,
                           ap=[[0, 128]] + list(g3_d.ap[1:]))
        nc.gpsimd.dma_start(out=g3_sb[:], in_=g3_bcast)
        g3v = g3_sb[:].rearrange("p (i k) -> p i k", k=K16)

        # broadcast x along the 16-knot window by log-doubling copies
        xt = pool.tile([BPC, IN_DIM, K16], F32)
        nc.vector.tensor_copy(xt[:, :, 0:1],
                              x_sb[:].rearrange("p (i k) -> p i k", k=1))
        w = 1
        while w < K16:
            n = min(w, K16 - w)
            nc.vector.tensor_copy(xt[:, :, w:w + n], xt[:, :, 0:n])
            w += n

        halves = ctx.enter_context(tc.tile_pool(name="halves", bufs=2))
        basesT = pool.tile([128, NG * 128], F32)
        HW_IN = IN_DIM // 2                       # 32 inputs per half
        for H in range(2):
            isl = slice(H * HW_IN, (H + 1) * HW_IN)
            Ah = halves.tile([BPC, HW_IN, K16], F32)
            nc.vector.tensor_sub(Ah[:], xt[:, isl, :], g3v[:, isl, :])
            Bab = halves.tile([BPC, HW_IN, 13], F32)
            nc.scalar.activation(out=Bab[:], in_=Ah[:, :, 1:14],
                                 func=mybir.ActivationFunctionType.Abs)
            B1h = halves.tile([BPC, HW_IN, 13], F32)
            # Relu(-8*|A| + 1) == Relu(1 - |A|/h)
            nc.scalar.activation(out=B1h[:], in_=Bab[:],
                                 func=mybir.ActivationFunctionType.Relu,
                                 scale=-8.0, bias=1.0)
            Ml2 = halves.tile([BPC, HW_IN, 12], F32)
            Mr2 = halves.tile([BPC, HW_IN, 12], F32)
            B2h = halves.tile([BPC, HW_IN, 12], F32)
            nc.vector.tensor_mul(Ml2[:], Ah[:, :, 0:12], B1h[:, :, 0:12])
            nc.vector.tensor_mul(Mr2[:], Ah[:, :, 3:15], B1h[:, :, 1:13])
            nc.vector.tensor_sub(B2h[:], Ml2[:], Mr2[:])
            Ml3 = halves.tile([BPC, HW_IN, 11], F32)
            Mr3 = halves.tile([BPC, HW_IN, 11], F32)
            B3h = halves.tile([BPC, HW_IN, K16], F32)
            nc.vector.tensor_mul(Ml3[:], Ah[:, :, 0:11], B2h[:, :, 0:11])
            nc.vector.tensor_mul(Mr3[:], Ah[:, :, 4:15], B2h[:, :, 1:12])
            # pad knots 11..15 must be 0: they feed the transpose, whose
            # output multiplies real coef columns.
            nc.gpsimd.memset(B3h[:, :, 11:16], 0.0)
            nc.vector.tensor_sub(B3h[:, :, 0:11], Ml3[:], Mr3[:])

            B3f = B3h[:].rearrange("p i k -> p (i k)")
            ps_t = psT.tile([128, 512], F32)
            for q in range(4):
                nc.tensor.transpose(out=ps_t[:, q * 128:(q + 1) * 128],
                                    in_=B3f[:, q * 128:(q + 1) * 128],
                                    identity=ident[:])
            dst = basesT[:, H * 512:(H + 1) * 512]
            if H == 0:
                nc.vector.tensor_copy(dst, ps_t[:])
            else:
                nc.scalar.copy(dst, ps_t[:])

            for q in range(4):
                g = 4 * H + q
                ps_o = psO.tile([128, 512], F32)
                nc.tensor.matmul(out=ps_o[:],
                                 lhsT=basesT[:, g * 128:(g + 1) * 128],
                                 rhs=rhs_sb[:, g * 512:(g + 1) * 512],
                                 start=True, stop=True)
                og = og_pool.tile([128, 512], F32)
                if g % 2 == 0:
                    nc.vector.tensor_copy(og[:], ps_o[:])
                else:
                    nc.scalar.copy(og[:], ps_o[:])
                nc.sync.dma_start(out=out_d[:, g, :], in_=og[:])

    nc.compile()
    return nc


def _host_inputs(x, coef, grid):
    x = np.ascontiguousarray(np.asarray(x, dtype=np.float32))
    coef = np.asarray(coef, dtype=np.float32)
    knots = np.asarray(grid, dtype=np.float32)[0, 0, :]          # (23,)
    h = float(knots[1] - knots[0])

    g3 = np.empty(K16, dtype=np.float32)
    g3[:15] = knots[8:23]
    g3[15] = knots[22] + h                                       # unused pad
    g3row = np.tile(g3, IN_DIM)[None, :]                         # (1, 1024)

    scale = 1.0 / (6.0 * h * h)
    cf = coef[:, :, 8:19] * scale                                # (o, i, 11)
    # block-diagonal rhs per group: rows (i_l,j) x cols (i_l', o), K=128, N=512
    rhs = np.zeros((128, NG * 512), dtype=np.float32)
    for i_l in range(8):
        for g in range(NG):
            i = g * 8 + i_l
            rhs[i_l * 16:i_l * 16 + 11,
                g * 512 + i_l * 64:g * 512 + i_l * 64 + 64] = cf[:, i, :].T
    return x, rhs, g3row


def _execute(x, coef, grid, trace=False, **spmd_kwargs):
    xf, rhs, g3row = _host_inputs(x, coef, grid)
    if "nc" not in _CACHE:
        _CACHE["nc"] = _build_nc()
    nc = _CACHE["nc"]
    in_maps = [{"x_in": np.ascontiguousarray(xf[c * BPC:(c + 1) * BPC]),
                "rhs_in": rhs, "g3_in": g3row} for c in range(N_CORES)]
    res = run_bass_kernel_spmd(nc, in_maps, list(range(N_CORES)),
                               trace=trace, **spmd_kwargs)
    full = np.empty((B_TOT, OUT_DIM, IN_DIM), dtype=np.float32)
    for c in range(N_CORES):
        t = res.results[c]["out"].reshape(BPC, NG, 8, 64)        # (b, g, i_l, o)
        full[c * BPC:(c + 1) * BPC] = (
            t.transpose(0, 3, 1, 2).reshape(BPC, OUT_DIM, IN_DIM))
    return full, res


def kernel(x, coef, grid):
    out, _ = _execute(x, coef, grid, trace=False)
    return out
